# revision 8
# baseline (speedup 1.0000x reference)
"""Multi-head attention (unstabilized softmax) on 8 TRN2 NeuronCores.

Reference computes, per (batch, head):
    scores  = Q @ K^T / sqrt(d)          [S, S]
    weights = exp(scores) / rowsum(exp)  (unstabilized softmax)
    out     = weights @ V                [S, d]

Sharding: B*H = 64 (batch, head) pairs split across 8 cores -> 8 heads per
core, fully independent (no collectives).

Device algorithm per head (S=2048, d=128):
  mm1: scoresT[k, q] = sum_d K[k,d] Q[q,d] with d on partitions.
       lhsT = K^T chunk [d=128, 128k] (stationary), rhs = Q^T [d=128, q]
       (host supplies Q^T/K^T layout [d, S]; bf16 matmul by default,
       ATTN_MM1_F32R=1 switches to fp32r for extra mm1 precision).
  exp: hybrid ACT + DVE.  ACT reads scoresT tiles from PSUM, computes
       exp(scale*x), writes bf16 W^T tiles to SBUF.  ACT alone is the
       bottleneck (1 elem/lane/cycle @1.2GHz + ~360ns/call fixed cost =
       ~312us/core), so a fraction of k-tiles is offloaded to the idle
       DVE via a one-op Schraudolph bit-trick: bf16(exp(z)) bits =
       int16(round(z*scale*2^23/(ln2*2^16) + (127*2^23 - C)/2^16)), i.e.
       a single tensor_scalar(mult,add) with int16 output viewed as bf16
       (~1.8% rms weight error on those tiles; rel-err budget 2e-2).
  mm2: out[q, 0:128] = sum_k W^T[k,q] * V[k,d];  col 128 = rowsum via a
       ones-column appended to V.  lhsT = W^T chunk (stationary, bf16+FWL),
       rhs = [V | 1] chunk [128k, 129].
  epilogue: DVE reciprocal of rowsum col + per-partition scalar multiply,
       DMA out f32.
"""

import math
import os

import numpy as np

import concourse.bass as bass  # noqa: F401  (bass types used via APs)
import concourse.mybir as mybir
from concourse import bacc
from concourse.tile import TileContext
from concourse.bass_utils import run_bass_kernel_spmd

B, H, S, D = 4, 16, 2048, 128
N_CORES = 8
HPC = (B * H) // N_CORES  # heads per core
SCALE = 1.0 / math.sqrt(D)

# Schraudolph exp constants for the DVE offload path (int16 == bf16 bits).
_SCH_C = 486411.0  # zero-mean tuning of the sawtooth error
SCH_A16 = (2.0**23) / math.log(2.0) / 65536.0 * SCALE
SCH_B16 = (127.0 * 2.0**23 - _SCH_C) / 65536.0

LAST_EXEC_TIME_NS = None
LAST_RESULTS = None
_NC_CACHE = {}


def build(hpc=HPC, s=S, mm1_f32r=True, exp_width=1024, dve_nkt=6, sch_b_off=0.0):
    """Build the per-core Bass graph. All cores run the same graph.

    dve_nkt: how many of the 16 k-tiles per period get their exp computed
    on DVE via the Schraudolph bit-trick instead of ACT (load balancing).
    """
    f32 = mybir.dt.float32
    f32r = mybir.dt.float32r
    bf16 = mybir.dt.bfloat16
    i16 = mybir.dt.int16

    ktn = s // 128   # number of 128-row k tiles
    ew = min(exp_width, s)  # exp/activation tile width (PSUM-resident scores)
    assert ew % 512 == 0 and s % ew == 0
    ewn = s // ew

    # Evenly-spread k-tiles whose exp goes to DVE (rest go to ACT).
    dve_kts = {(i * ktn) // dve_nkt for i in range(dve_nkt)} if dve_nkt else set()
    sch_b = SCH_B16 + sch_b_off

    nc = bacc.Bacc(None, target_bir_lowering=False)

    qk_dt = f32r if mm1_f32r else f32
    qt_d = nc.declare_dram_parameter("qt", [hpc, D, s], qk_dt, isOutput=False)
    kt_d = nc.declare_dram_parameter("kt", [hpc, D, s], qk_dt, isOutput=False)
    v_d = nc.declare_dram_parameter("v", [hpc, s, D], f32, isOutput=False)
    o_d = nc.declare_dram_parameter("out", [hpc, s, D], f32, isOutput=True)

    qh_n = hpc * ewn  # total q-half periods (pipeline granularity)

    with TileContext(nc) as tc:
        with (
            tc.tile_pool(name="qkstage", bufs=1) as qkstage_pool,
            tc.tile_pool(name="qk", bufs=2) as qk_pool,
            tc.tile_pool(name="vio", bufs=2) as vio_pool,
            tc.tile_pool(name="vaug", bufs=2) as vaug_pool,
            tc.tile_pool(name="wt", bufs=3) as wt_pool,
            tc.tile_pool(name="osb", bufs=4) as osb_pool,
            tc.tile_pool(name="rc", bufs=4) as rc_pool,
            tc.tile_pool(name="scoreps", bufs=3, space="PSUM") as score_pool,
            tc.tile_pool(name="outps", bufs=2, space="PSUM") as out_ps_pool,
        ):
            head_state = {}

            def load_head(h):
                """DMA + cast head h inputs; returns (q_mm, k_mm, v_aug)."""
                if mm1_f32r:
                    q_mm = qk_pool.tile([128, s], qk_dt, tag="q")
                    k_mm = qk_pool.tile([128, s], qk_dt, tag="k")
                    nc.sync.dma_start(out=q_mm, in_=qt_d[h])
                    nc.sync.dma_start(out=k_mm, in_=kt_d[h])
                else:
                    q_sb = qkstage_pool.tile([128, s], f32, tag="qs")
                    k_sb = qkstage_pool.tile([128, s], f32, tag="ks")
                    q_mm = qk_pool.tile([128, s], bf16, tag="q")
                    k_mm = qk_pool.tile([128, s], bf16, tag="k")
                    # Chunked DMA+cast so the first mm1 (low k/q columns) can
                    # start before the full tensors land. Head 0 gates the
                    # whole pipeline, so chunk it finest (startup latency).
                    nch = 4 if h == 0 else 2
                    cs = s // nch
                    for ci in range(nch):
                        c0 = ci * cs
                        nc.sync.dma_start(
                            out=k_sb[:, c0 : c0 + cs], in_=kt_d[h, :, c0 : c0 + cs]
                        )
                        nc.vector.tensor_copy(
                            out=k_mm[:, c0 : c0 + cs], in_=k_sb[:, c0 : c0 + cs]
                        )
                        nc.sync.dma_start(
                            out=q_sb[:, c0 : c0 + cs], in_=qt_d[h, :, c0 : c0 + cs]
                        )
                        nc.vector.tensor_copy(
                            out=q_mm[:, c0 : c0 + cs], in_=q_sb[:, c0 : c0 + cs]
                        )

                v_sb = vio_pool.tile([128, ktn, D], f32, tag="v")
                nc.sync.dma_start(
                    out=v_sb, in_=v_d[h].rearrange("(kt p) d -> p kt d", p=128)
                )
                v_aug = vaug_pool.tile([128, ktn, D + 1], bf16, tag="vaug")
                # GPSIMD (otherwise idle) prepares [V | 1]; frees DVE for exp.
                nc.gpsimd.memset(v_aug[:, :, D : D + 1], 1.0)
                nc.gpsimd.tensor_copy(out=v_aug[:, :, 0:D], in_=v_sb)
                return q_mm, k_mm, v_aug

            def mm1_exp(per, filler=()):
                """mm1 + exp for q-half period `per`; returns wt half tile.

                `filler` is a list of closures (pending mm2 q-tile emitters)
                interleaved one-per-kt so the PE stream has mm2 work while it
                is PSUM-throttled behind ACT.
                """
                h, ei = divmod(per, ewn)
                if per == 0:
                    head_state[0] = load_head(0)
                # Prefetch the next head's load+cast one period before it is
                # consumed: gives the DMA a full period of slack so the casts
                # never head-of-line-block the DVE epilogue ops behind them.
                nxt_per = per + 1
                if nxt_per < qh_n and nxt_per % ewn == 0:
                    nh = nxt_per // ewn
                    if nh not in head_state:
                        head_state[nh] = load_head(nh)
                q_mm, k_mm, _ = head_state[h]
                wt = wt_pool.tile([128, ktn, ew], bf16, tag="wt")
                fill_iter = iter(filler)
                for kt in range(ktn):
                    ps = score_pool.tile([128, ew], f32, tag="score")
                    for sub in range(ew // 512):
                        q0 = ei * ew + sub * 512
                        nc.tensor.matmul(
                            out=ps[:, sub * 512 : (sub + 1) * 512],
                            lhsT=k_mm[:, kt * 128 : (kt + 1) * 128],
                            rhs=q_mm[:, q0 : q0 + 512],
                            start=True,
                            stop=True,
                        )
                    if kt in dve_kts:
                        # exp on DVE: affine in f32, converted to int16 whose
                        # bits are the bf16 of exp(SCALE*score).
                        nc.vector.tensor_scalar(
                            out=wt[:, kt, :].bitcast(i16),
                            in0=ps,
                            scalar1=SCH_A16,
                            scalar2=sch_b,
                            op0=mybir.AluOpType.mult,
                            op1=mybir.AluOpType.add,
                        )
                    else:
                        nc.scalar.activation(
                            out=wt[:, kt, :],
                            in_=ps,
                            func=mybir.ActivationFunctionType.Exp,
                            scale=SCALE,
                        )
                    nxt = next(fill_iter, None)
                    if nxt is not None:
                        nxt()
                for nxt in fill_iter:
                    nxt()
                return wt

            def emit_mm2_qtile(per, wt, qi):
                """mm2 + normalize + store for one 128-row q tile."""
                h, ei = divmod(per, ewn)
                v_aug = head_state[h][2]
                qg = ei * (ew // 128) + qi  # global q-tile in head
                po = out_ps_pool.tile([128, D + 1], f32, tag="po")
                for kt in range(ktn):
                    nc.tensor.matmul(
                        out=po,
                        lhsT=wt[:, kt, qi * 128 : (qi + 1) * 128],
                        rhs=v_aug[:, kt, :],
                        start=(kt == 0),
                        stop=(kt == ktn - 1),
                    )
                rc = rc_pool.tile([128, 1], f32, tag="rc")
                nc.vector.reciprocal(out=rc, in_=po[:, D : D + 1])
                o_sb = osb_pool.tile([128, D], f32, tag="osb")
                # Load-balance the normalize multiply: every 4th q-tile goes
                # to ACT (activation Copy with per-partition AP scale).
                if qg % 4 == 3:
                    nc.scalar.mul(o_sb, po[:, 0:D], rc)
                else:
                    nc.vector.tensor_scalar_mul(o_sb, po[:, 0:D], rc)
                # Last head's stores go on the (by-then idle) HWDGE sync queue
                # instead of SWDGE: SWDGE first-byte latency sits on the tail.
                store_eng = nc.sync if h == hpc - 1 else nc.gpsimd
                store_eng.dma_start(out=o_d[h, qg * 128 : (qg + 1) * 128, :], in_=o_sb)

            def mm2_epilogue(per, wt):
                for qi in range(ew // 128):
                    emit_mm2_qtile(per, wt, qi)

            # Software pipeline over q-half periods, distance 2: emit period
            # i's mm1/exp before period (i-2)'s mm2 so the in-order PE stream
            # always has matmul work while ACT catches up on exp.
            state = {}
            for per in range(qh_n):
                state[per] = mm1_exp(per)
                if per >= 2:
                    mm2_epilogue(per - 2, state.pop(per - 2))
            for p in sorted(state):
                mm2_epilogue(p, state.pop(p))

    return nc


def _shard_host(Q, K, V, hpc, n_cores):
    """Host-side shard + layout: returns per-core input maps."""
    BH = Q.shape[0] * Q.shape[1]
    s, d = Q.shape[2], Q.shape[3]
    Qf = np.ascontiguousarray(Q.reshape(BH, s, d))
    Kf = np.ascontiguousarray(K.reshape(BH, s, d))
    Vf = np.ascontiguousarray(V.reshape(BH, s, d))
    in_maps = []
    for c in range(n_cores):
        sl = slice(c * hpc, (c + 1) * hpc)
        in_maps.append(
            {
                "qt": np.ascontiguousarray(Qf[sl].transpose(0, 2, 1)),
                "kt": np.ascontiguousarray(Kf[sl].transpose(0, 2, 1)),
                "v": Vf[sl],
            }
        )
    return in_maps


def kernel(Q, K, V):
    global LAST_EXEC_TIME_NS, LAST_RESULTS
    Q = np.asarray(Q, dtype=np.float32)
    K = np.asarray(K, dtype=np.float32)
    V = np.asarray(V, dtype=np.float32)

    mm1_f32r = os.environ.get("ATTN_MM1_F32R", "0") == "1"
    trace = os.environ.get("ATTN_TRACE", "0") == "1"
    dve_nkt = int(os.environ.get("ATTN_DVE_NKT", "6"))
    sch_b_off = float(os.environ.get("ATTN_SCH_B_OFF", "0.0"))

    key = (HPC, S, mm1_f32r, dve_nkt, sch_b_off)
    nc = _NC_CACHE.get(key)
    if nc is None:
        nc = build(hpc=HPC, s=S, mm1_f32r=mm1_f32r, dve_nkt=dve_nkt,
                   sch_b_off=sch_b_off)
        nc.compile()
        _NC_CACHE[key] = nc

    in_maps = _shard_host(Q, K, V, HPC, N_CORES)
    res = run_bass_kernel_spmd(nc, in_maps, core_ids=list(range(N_CORES)), trace=trace)
    LAST_EXEC_TIME_NS = res.exec_time_ns
    LAST_RESULTS = res

    out = np.concatenate([res.results[c]["out"] for c in range(N_CORES)], axis=0)
    return np.ascontiguousarray(out.reshape(B, H, S, D))

